# revision 1
# baseline (speedup 1.0000x reference)
"""LocallyConnected1d Trainium2 kernel (8 NeuronCores, sequence-parallel).

Problem: out[b,o,l] = sum_{i,k} xpad[b,i,l+k] * w[i,o,k,l] + bias[o,l]
  B=64, Ci=Co=64, S=L=512, K=9, pad=4.

Strategy:
  * Shard out_seq_len L=512 across 8 cores (64 positions each) so the 75MB
    per-position weight tensor is moved from HBM exactly once (weight DMA is
    the roofline: ~4.7MB/core in bf16).
  * Per core, process positions in pairs (l, l+1). Contract dim is laid out
    as r = dj*64 + i (dj in {0,1}), split into 5 chunks c, where chunk c
    covers window offsets j = 2c+dj of the padded input.
  * matmul: stationary lhsT = X block [128=(dj,i), 64=b] (cheap 64-col
    LDWEIGHTS), moving rhs = weight block [128=(dj,i), 128=(l2,o)], PSUM
    out [64=b, 128=(l2,o)] accumulates over the 5 chunks.
    Weight entry at (dj,i),(l2,o) of chunk c is w[i,o,2c+dj-l2, l+l2]
    (zero if k=2c+dj-l2 outside [0,9)) -- so one pair of output positions
    is computed per PSUM tile with zero wasted streaming columns.
  * bias is folded in as a 6th rank-1 matmul: ones[1,64].T @ bias_row[1,128].
  * All operands bf16 (halves DMA + enables fast PE streaming), PSUM fp32.
"""

import sys

sys.path.insert(0, "/opt/trn_rl_repo")

import numpy as np
from ml_dtypes import bfloat16

import concourse.bass as bass
import concourse.bacc as bacc
import concourse.mybir as mybir
from concourse import tile
from concourse.bass_utils import run_bass_kernel_spmd

B = 64
CI = 64
CO = 64
S = 512
KS = 9
PAD = 4
L = 512
NCORES = 8
LS = L // NCORES          # 64 output positions per core
NPAIR = LS // 2           # 32 position pairs per core
NCH = 5                   # contract chunks per pair (j window of 10 -> 5x128)
NT = LS // 2 + NCH - 1    # 36 x-blocks of [128, 64]
OUT_GROUPS = 4            # output DMA granularity (8 pairs each)

TRACE = False
TRACE_KW: dict = {}
LAST_RESULT = None

_cached_nc = None


def _build_nc():
    global _cached_nc
    if _cached_nc is not None:
        return _cached_nc

    nc = bacc.Bacc("TRN2", target_bir_lowering=False, debug=False,
                   num_devices=NCORES)
    bf = mybir.dt.bfloat16
    f32 = mybir.dt.float32

    xs_d = nc.dram_tensor("xs", [128, NT * 64], bf, kind="ExternalInput").ap()
    ws_d = nc.dram_tensor("ws", [NPAIR, 128, NCH * 128], bf,
                          kind="ExternalInput").ap()
    bs_d = nc.dram_tensor("bs", [1, NPAIR * 128], bf, kind="ExternalInput").ap()
    out_d = nc.dram_tensor("out", [64, NPAIR * 128], f32,
                           kind="ExternalOutput").ap()

    with tile.TileContext(nc) as tc:
        with (
            tc.tile_pool(name="xp", bufs=1) as xp,
            tc.tile_pool(name="wp", bufs=NPAIR) as wp,
            tc.tile_pool(name="pp", bufs=8, space="PSUM") as pp,
            tc.tile_pool(name="op", bufs=OUT_GROUPS) as op,
        ):
            xs_t = xp.tile([128, NT * 64], bf, tag="xs")
            nc.sync.dma_start(xs_t[:], xs_d[:])
            bs_t = xp.tile([1, NPAIR * 128], bf, tag="bs")
            nc.sync.dma_start(bs_t[:], bs_d[:])
            ones_t = xp.tile([1, 64], bf, tag="ones")
            nc.gpsimd.memset(ones_t[:], 1.0)

            w_tiles = []
            for p in range(NPAIR):
                wt = wp.tile([128, NCH * 128], bf, tag="wt")
                nc.sync.dma_start(wt[:], ws_d[p])
                w_tiles.append(wt)

            ppg = NPAIR // OUT_GROUPS
            for g in range(OUT_GROUPS):
                ot = op.tile([64, ppg * 128], f32, tag="ot")
                for pp_i in range(ppg):
                    p = g * ppg + pp_i
                    ps = pp.tile([64, 128], f32, tag="ps")
                    for c in range(NCH):
                        nc.tensor.matmul(
                            ps[:],
                            xs_t[:, (p + c) * 64:(p + c + 1) * 64],
                            w_tiles[p][:, c * 128:(c + 1) * 128],
                            start=(c == 0),
                            stop=False,
                        )
                    nc.tensor.matmul(
                        ps[:],
                        ones_t[:],
                        bs_t[:, p * 128:(p + 1) * 128],
                        start=False,
                        stop=True,
                    )
                    nc.vector.tensor_copy(ot[:, pp_i * 128:(pp_i + 1) * 128],
                                          ps[:])
                nc.sync.dma_start(
                    out_d[:, g * ppg * 128:(g + 1) * ppg * 128], ot[:])

    nc.compile()
    _cached_nc = nc
    return nc


def _prep_core_inputs(xpad, weight, bias, cr):
    l0 = LS * cr
    # xs[dj*64+i, t*64+b] = xpad[b, i, l0+2t+dj]
    xsl = xpad[:, :, l0:l0 + 2 * NT]                       # [b, i, 72]
    xs = np.ascontiguousarray(
        xsl.reshape(B, CI, NT, 2).transpose(3, 1, 2, 0)    # [dj, i, t, b]
    ).reshape(128, NT * 64)

    # ws[p, dj*64+i, c*128 + l2*64 + o] = w[i,o,2c+dj-l2, l0+2p+l2]
    wsarr = np.zeros((NPAIR, 2, CI, NCH, 2, CO), np.float32)
    for c in range(NCH):
        for dj in range(2):
            for l2 in range(2):
                k = 2 * c + dj - l2
                if 0 <= k < KS:
                    wsl = weight[:, :, k, l0 + l2:l0 + l2 + 64:2]  # [i,o,p]
                    wsarr[:, dj, :, c, l2, :] = wsl.transpose(2, 0, 1)
    ws = wsarr.reshape(NPAIR, 128, NCH * 128)

    # bs[0, p*128 + l2*64 + o] = bias[o, l0+2p+l2]
    bs = np.ascontiguousarray(
        bias[:, l0:l0 + LS].reshape(CO, NPAIR, 2).transpose(1, 2, 0)
    ).reshape(1, NPAIR * 128)

    return {
        "xs": xs.astype(bfloat16),
        "ws": ws.astype(bfloat16),
        "bs": bs.astype(bfloat16),
    }


def kernel(x, weight, bias):
    global LAST_RESULT
    x = np.asarray(x, np.float32)
    weight = np.asarray(weight, np.float32)
    bias = np.asarray(bias, np.float32)

    nc = _build_nc()

    xpad = np.zeros((B, CI, S + 2 * PAD), np.float32)
    xpad[:, :, PAD:PAD + S] = x

    in_maps = [_prep_core_inputs(xpad, weight, bias, cr)
               for cr in range(NCORES)]

    kw = dict(TRACE_KW)
    if TRACE:
        kw.setdefault("trace", True)
    res = run_bass_kernel_spmd(nc, in_maps, list(range(NCORES)), **kw)
    LAST_RESULT = res

    out = np.empty((B, CO, L), np.float32)
    for cr in range(NCORES):
        r = np.asarray(res.results[cr]["out"], np.float32)   # [64, 4096]
        out[:, :, LS * cr:LS * (cr + 1)] = (
            r.reshape(B, NPAIR, 2, CO).transpose(0, 3, 1, 2).reshape(B, CO, LS)
        )
    return out
